# revision 18
# baseline (speedup 1.0000x reference)
"""Distributed Bass kernel for attention-energy softmax on 8 TRN2 NeuronCores.

Computes: softmax(enc @ W.T @ h + (b.h)) == softmax(enc @ (W.T @ h)) over S=32768.
The bias term b.h is a constant shift across all energies and cancels in softmax,
so b is unused.

Sharding: encoder_output split along S into 8 shards of 4096 rows; each shard is
host-transposed to [H, S_shard] and cast to fp16 so the contraction dim (H)
lands on SBUF partitions and DMA/TensorE run at 16-bit rates. W and h are
replicated fp16. fp16 products accumulate exactly in fp32 PSUM; the softmax
rel err of the fp16 path is ~6e-3 (measured), well under the 2e-2 gate.

Per core:
  v_row[1,1024] = h-chunk-stationary @ Wh (moving, N=512)   16 matmuls
  v_col[128,8]  = per-chunk PE transpose of v_row (outer product with [1,1])
  e[1,4096]     = sum_hc vh_col[:,hc].T @ enc_slab_hc        64 matmuls (M=1,
                  N=512) into ONE PSUM tensor spanning all 8 banks
  local stats in two ops (one reduce_max, one Exp with accum_out),
  AllGather of (m_loc, S_loc), single-scalar rescale of the saved exp values.
"""

import sys

sys.path.insert(0, "/opt/trn_rl_repo")

import numpy as np

import concourse.bacc as bacc
import concourse.mybir as mybir
import concourse.tile as tile
from concourse.bass_utils import run_bass_kernel_spmd

N_CORES = 8
H = 1024
S = 32768
S_SHARD = S // N_CORES          # 4096
HC = H // 128                   # 8 h-chunks of 128 (contraction tiles)
NB = S_SHARD // 512             # 8 PSUM-bank-sized energy slices
FP32 = mybir.dt.float32
FP16 = mybir.dt.float16
RG = [list(range(N_CORES))]

_compiled_nc = None


def _build():
    nc = bacc.Bacc(
        "TRN2", target_bir_lowering=False, debug=False, num_devices=N_CORES
    )

    encT = nc.dram_tensor("encT", [H, S_SHARD], FP16, kind="ExternalInput")
    hh2 = nc.dram_tensor("hh2", [128, HC], FP16, kind="ExternalInput")
    Wh = nc.dram_tensor("Wh", [H, H], FP16, kind="ExternalInput")
    out_ext = nc.dram_tensor("out", [1, S_SHARD], FP32, kind="ExternalOutput")

    EXP = mybir.ActivationFunctionType.Exp
    AX = mybir.AxisListType.X

    with tile.TileContext(nc) as tc:
        with (
            tc.tile_pool(name="sb", bufs=1) as sb,
            tc.tile_pool(name="enc", bufs=9) as encp,
            tc.tile_pool(name="dram", bufs=1, space="DRAM") as dramp,
        ):
            # --- small inputs / constants ---
            Wh_sb = sb.tile([128, HC * H], FP16, tag="Wh")
            hh_sb = sb.tile([128, HC], FP16, tag="hh")
            one1 = sb.tile([1, 1], FP32, tag="one1")

            nc.sync.dma_start(out=hh_sb[:, :], in_=hh2[:, :])
            nc.sync.dma_start(
                out=Wh_sb[:, :].rearrange("p (c j) -> p c j", c=HC),
                in_=Wh[:, :].rearrange("(c p) j -> p c j", p=128),
            )
            nc.vector.memset(one1[:, :], 1.0)

            # --- v phase: v_row[0, j] = v[j] = sum_k W[k, j] h[k] ---
            v_row_sb = sb.tile([1, H], FP32, tag="vrow")
            vh_col = sb.tile([128, HC], FP16, tag="vhcol")
            with tc.tile_pool(name="psv", bufs=1, space="PSUM") as psv:
                v_row_ps = psv.tile([1, H], FP32, tag="vrps")
                for jb in range(H // 512):
                    for kc in range(HC):
                        nc.tensor.matmul(
                            v_row_ps[0:1, jb * 512 : (jb + 1) * 512],
                            lhsT=hh_sb[:, kc : kc + 1],
                            rhs=Wh_sb[
                                :, kc * H + jb * 512 : kc * H + jb * 512 + 512
                            ],
                            start=(kc == 0),
                            stop=(kc == HC - 1),
                        )
                nc.vector.tensor_copy(v_row_sb[:, :], v_row_ps[:, :])
                # transpose v chunks onto partitions via outer product w/ [1,1]
                v_col_ps = psv.tile([128, HC], FP32, tag="vcps")
                for hc in range(HC):
                    nc.tensor.matmul(
                        v_col_ps[:, hc : hc + 1],
                        lhsT=v_row_sb[0:1, hc * 128 : (hc + 1) * 128],
                        rhs=one1[0:1, 0:1],
                        start=True,
                        stop=True,
                    )
                nc.vector.tensor_copy(vh_col[:, :], v_col_ps[:, :])  # cast f16

            # --- e phase: one [1, 4096] PSUM tensor spanning all 8 banks ---
            scratch = sb.tile([1, S_SHARD], FP32, tag="scr")
            m_loc = sb.tile([1, 1], FP32, tag="mloc")
            ngl = sb.tile([1, 1], FP32, tag="ngl")
            S_loc = sb.tile([1, 1], FP32, tag="Sloc")
            with tc.tile_pool(name="pse", bufs=1, space="PSUM") as pse:
                e_ps = pse.tile([1, S_SHARD], FP32, tag="eps")
                for hc in range(HC):
                    slab = encp.tile([128, S_SHARD], FP16, tag="slab")
                    nc.sync.dma_start(
                        out=slab[:, :], in_=encT[hc * 128 : (hc + 1) * 128, :]
                    )
                    for b in range(NB):
                        nc.tensor.matmul(
                            e_ps[0:1, b * 512 : (b + 1) * 512],
                            lhsT=vh_col[:, hc : hc + 1],
                            rhs=slab[:, b * 512 : (b + 1) * 512],
                            start=(hc == 0),
                            stop=(hc == HC - 1),
                        )
                # local stats: one max, one exp-with-accumulate
                nc.vector.reduce_max(m_loc[:, :], e_ps[0:1, :], axis=AX)
                nc.vector.tensor_scalar_mul(ngl[:, :], m_loc[:, :], -1.0)
                nc.scalar.activation(
                    scratch[0:1, :], e_ps[0:1, :], EXP,
                    bias=ngl[0:1, 0:1], scale=1.0, accum_out=S_loc[:, :],
                )

            # --- exchange (m_loc, S_loc) across cores ---
            stats_sb = sb.tile([1, 2], FP32, tag="stats")
            nc.vector.tensor_copy(stats_sb[0:1, 0:1], m_loc[0:1, 0:1])
            nc.vector.tensor_copy(stats_sb[0:1, 1:2], S_loc[0:1, 0:1])

            stats_d = dramp.tile([1, 2], FP32, tag="statsd")
            gath_d = dramp.tile([N_CORES, 2], FP32, tag="gathd")
            nc.sync.dma_start(out=stats_d[:, :], in_=stats_sb[0:1, :])
            nc.gpsimd.collective_compute(
                "AllGather",
                mybir.AluOpType.bypass,
                replica_groups=RG,
                ins=[stats_d.opt()],
                outs=[gath_d.opt()],
            )
            gath_sb = sb.tile([1, 2 * N_CORES], FP32, tag="gath")
            nc.sync.dma_start(
                out=gath_sb[0:1, :], in_=gath_d[:, :].rearrange("a b -> (a b)")
            )

            # --- global combine on partition 0 ---
            ms = gath_sb[0:1, 0 : 2 * N_CORES : 2]
            ss = gath_sb[0:1, 1 : 2 * N_CORES : 2]
            M_g = sb.tile([1, 1], FP32, tag="Mg")
            ngM = sb.tile([1, 1], FP32, tag="ngM")
            t8 = sb.tile([1, N_CORES], FP32, tag="t8")
            z8 = sb.tile([1, N_CORES], FP32, tag="z8")
            Z_g = sb.tile([1, 1], FP32, tag="Zg")
            rZ = sb.tile([1, 1], FP32, tag="rZ")
            r1 = sb.tile([1, 1], FP32, tag="r1")
            sc1 = sb.tile([1, 1], FP32, tag="sc1")
            nc.vector.reduce_max(M_g[:, :], ms, axis=AX)
            nc.vector.tensor_scalar_mul(ngM[:, :], M_g[:, :], -1.0)
            nc.scalar.activation(t8[0:1, :], ms, EXP, bias=ngM[0:1, 0:1])
            nc.vector.tensor_mul(z8[0:1, :], t8[0:1, :], ss)
            nc.vector.reduce_sum(Z_g[:, :], z8[0:1, :], axis=AX)
            nc.vector.reciprocal(rZ[:, :], Z_g[:, :])
            nc.scalar.activation(r1[0:1, :], m_loc[0:1, 0:1], EXP,
                                 bias=ngM[0:1, 0:1])
            nc.vector.tensor_mul(sc1[:, :], r1[:, :], rZ[:, :])

            # --- final rescale + store ---
            out_row = sb.tile([1, S_SHARD], FP32, tag="outr")
            nc.vector.tensor_scalar_mul(
                out_row[0:1, :], scratch[0:1, :], sc1[0:1, 0:1]
            )
            nc.sync.dma_start(out=out_ext[:, :], in_=out_row[0:1, :])

    nc.compile()
    return nc


def get_nc():
    global _compiled_nc
    if _compiled_nc is None:
        _compiled_nc = _build()
    return _compiled_nc


def make_in_maps(hidden_state, encoder_output, W):
    h = np.asarray(hidden_state, dtype=np.float32).reshape(H)
    enc = np.asarray(encoder_output, dtype=np.float32).reshape(S, H)
    Wf = np.asarray(W, dtype=np.float32).reshape(H, H)

    h2 = h.reshape(HC, 128).T  # h2[p, c] = h[c*128 + p]
    hh2 = np.ascontiguousarray(h2.astype(np.float16))
    Wh = np.ascontiguousarray(Wf.astype(np.float16))

    in_maps = []
    for c in range(N_CORES):
        shard = np.ascontiguousarray(
            enc[c * S_SHARD : (c + 1) * S_SHARD, :].T.astype(np.float16)
        )  # [H, S_SHARD] fp16
        in_maps.append({"encT": shard, "hh2": hh2, "Wh": Wh})
    return in_maps


def unshard(results):
    out = np.empty((1, S), dtype=np.float32)
    for c in range(N_CORES):
        out[0, c * S_SHARD : (c + 1) * S_SHARD] = results[c]["out"].reshape(
            S_SHARD
        )
    return out


def kernel(hidden_state, encoder_output, W, b=None, **_unused):
    nc = get_nc()
    in_maps = make_in_maps(hidden_state, encoder_output, W)
    res = run_bass_kernel_spmd(nc, in_maps, core_ids=list(range(N_CORES)))
    return unshard(res.results)
